# revision 35
# baseline (speedup 1.0000x reference)
"""Correlation network kernel for Trainium2.

corr[b,i,j,k,l] = sum_c A[b,i,j,c] * B[b,k,l,c]

Per batch b this is  A_b (2304x64) @ B_b^T (64x2304) -> 2304x2304.
Sharding: data-parallel over batch B=8 across the 8 NeuronCores; each core
computes one full 2304x2304 correlation matrix (21.2 MB fp32 out), so the
kernel is output-write bound (~358 GB/s HBM per core => ~60 us floor).

Device-side plan (per core):
  - Inputs arrive host-prepped: transposed to [C, HW] layout, bf16 hi/lo
    split. The kernel computes hi*hi + hi*lo (the lo*hi term is dropped)
    and emits the OUTPUT AS BF16, halving the dominant HBM write traffic
    (21.2 -> 10.6 MB/core; fro rel err ~2.3e-3, well inside the 2e-2
    gate; the host upcasts to fp32).
  - K=C=64 uses only half the 128-row PE array, so m-tiles are packed in
    pairs: even m-tiles occupy array rows 0-63, odd m-tiles rows 64-127.
    The two groups' matmuls run concurrently; B^T operands are duplicated
    into both partition halves so each group streams its own rows.
  - Inputs load via the two HWDGE rings (sync + scalar) right after the
    ~7 us Tile preamble; SWDGE/gpsimd was ~2 us slower to first byte.
  - Per (m-pair, 512-col n-tile): 4 bf16 matmuls into two PSUM banks,
    then narrow [128,512] PSUM->SBUF casts alternating between DVE and
    ACT (both engines ~3.2 us/pair -- the copy stream and the ~358 GB/s
    HBM write wire are the co-pacers of the kernel).
  - Output flushes: per m-tile one 512 KB chunk (cols 0:2048 = exactly
    4 KB/partition descriptors) + the 64 KB tail; even tiles on the SP
    ring, odd on the ACT ring so each engine issues one DMA per pair and
    waits only on its own side's copies.
"""

import numpy as np
import ml_dtypes

import concourse.bacc as bacc
import concourse.mybir as mybir
import concourse.tile as tile
from concourse.bass_interp import get_hw_module
from concourse.bass_utils import run_bass_kernel_spmd

B, H, W, C = 8, 48, 48, 64
HW = H * W  # 2304
P = 128
M_TILES = HW // P  # 18
M_PAIRS = M_TILES // 2  # 9
N_TILE = 512
FP32 = mybir.dt.float32
BF16 = mybir.dt.bfloat16
BF16_NP = ml_dtypes.bfloat16

N_SPLITS = []
_n0 = 0
while _n0 < HW:
    N_SPLITS.append((_n0, min(N_TILE, HW - _n0)))
    _n0 += N_TILE


def _corr_body(tc, out, a_hi, b_hi, b_lo):
    nc = tc.nc
    with (
        tc.tile_pool(name="ops", bufs=1) as op_pool,
        tc.tile_pool(name="ps", bufs=8, space="PSUM") as ps_pool,
        tc.tile_pool(name="outs", bufs=12) as out_pool,
    ):
        # lhsT operand: [128, 1152]; rows 0:64 = even m-tiles, 64:128 = odd
        ath = op_pool.tile([P, HW // 2], BF16)
        # rhs operands: [128, 2304]; rows 64:128 duplicate rows 0:64
        bth = op_pool.tile([P, HW], BF16)
        btl = op_pool.tile([P, HW], BF16)
        # Inputs ride the HWDGE rings (first byte ~0.6 us after the Tile
        # preamble vs ~9 us for the SWDGE/gpsimd path, and ~360 vs ~230
        # GB/s). Each dma_start costs ~0.6-0.7 us of issuing-engine time,
        # so split the issues across BOTH rings: sync takes the hh-term
        # operands (needed first), scalar takes the hl-term ones. Outputs
        # are issued later on the same rings (FIFO per ring, no conflict).
        # ALL inputs ride the scalar/ACT ring, issued before any copies
        # start; ALL outputs ride the sync/SP ring. The ACT engine then
        # carries ONLY the copy stream in steady state (it was the pacer
        # at ~3.9 us/pair with copies + per-pair DMA issues), and the
        # sync engine absorbs every output issue + its waits. The second
        # b chunks split at 1024 so ni=1's matmuls aren't gated on the
        # big-chunk completion receipt.
        for t, src, c0, c1 in [
            (ath, a_hi, 0, P),
            (bth, b_hi, 0, N_TILE),
            (btl, b_lo, 0, N_TILE),
            (bth, b_hi, N_TILE, 2 * N_TILE),
            (btl, b_lo, N_TILE, 2 * N_TILE),
            (ath, a_hi, P, HW // 2),
            (bth, b_hi, 2 * N_TILE, HW),
            (btl, b_lo, 2 * N_TILE, HW),
        ]:
            nc.scalar.dma_start(out=t[:, c0:c1], in_=src[:, c0:c1])

        for p in range(M_PAIRS):
            ot_e = out_pool.tile([P, HW], BF16, tag="ot")
            ot_o = out_pool.tile([P, HW], BF16, tag="ot")
            col = slice(p * P, (p + 1) * P)
            for ni, (n0, nsz) in enumerate(N_SPLITS):
                ps_e = ps_pool.tile([P, N_TILE], FP32, tag="ps")
                ps_o = ps_pool.tile([P, N_TILE], FP32, tag="ps")
                for k, (at, bt) in enumerate(((ath, bth), (ath, btl))):
                    st, sp = k == 0, k == 1
                    nc.tensor.matmul(
                        ps_e[:, :nsz],
                        at[0:64, col],
                        bt[0:64, n0 : n0 + nsz],
                        start=st,
                        stop=sp,
                    )
                    nc.tensor.matmul(
                        ps_o[:, :nsz],
                        at[64:128, col],
                        bt[64:128, n0 : n0 + nsz],
                        start=st,
                        stop=sp,
                    )
                # balance the narrow PSUM->SBUF copies across DVE and ACT
                # (narrow [128,512] copies keep the PSUM-recycle dependency
                # chain short; wide 2-bank copies measured slower overall)
                if ni % 2 == 0:
                    nc.vector.tensor_copy(ot_e[:, n0 : n0 + nsz], ps_e[:, :nsz])
                    nc.scalar.copy(ot_o[:, n0 : n0 + nsz], ps_o[:, :nsz])
                else:
                    nc.scalar.copy(ot_e[:, n0 : n0 + nsz], ps_e[:, :nsz])
                    nc.vector.tensor_copy(ot_o[:, n0 : n0 + nsz], ps_o[:, :nsz])
                # Flush 512 KB chunks (cols 0:2048 = exactly 4 KB/partition;
                # 4608 B patterns split 4096+512 and drop to ~205 GB/s) plus
                # the 64 KB tails. Even tiles drain on the SP ring, odd on
                # the ACT ring (a dma_start can only target the issuing
                # engine's own ring): each ring gets one DMA per pair
                # continuously, halving issue cost and blocking waits per
                # engine. The first pair flushes at ni=1 to start the
                # stream early.
                if p == 0:
                    sched = {1: 0, 3: 2 * N_TILE, 4: 4 * N_TILE}
                else:
                    sched = {3: 0, 4: 4 * N_TILE}
                if ni in sched:
                    c0 = sched[ni]
                    c1 = n0 + nsz
                    m_e, m_o = 2 * p, 2 * p + 1
                    nc.sync.dma_start(
                        out=out[m_e * P : (m_e + 1) * P, c0:c1],
                        in_=ot_e[:, c0:c1],
                    )
                    nc.sync.dma_start(
                        out=out[m_o * P : (m_o + 1) * P, c0:c1],
                        in_=ot_o[:, c0:c1],
                    )


_NC_CACHE = None


def _build():
    global _NC_CACHE
    if _NC_CACHE is None:
        nc = bacc.Bacc(
            "TRN2",
            target_bir_lowering=False,
            debug=False,
            enable_asserts=False,
        )
        a_hi = nc.dram_tensor("a_hi", [P, HW // 2], BF16, kind="ExternalInput").ap()
        b_hi = nc.dram_tensor("b_hi", [P, HW], BF16, kind="ExternalInput").ap()
        b_lo = nc.dram_tensor("b_lo", [P, HW], BF16, kind="ExternalInput").ap()
        out = nc.dram_tensor("out", [HW, HW], BF16, kind="ExternalOutput").ap()
        with tile.TileContext(nc) as tc:
            _corr_body(tc, out, a_hi, b_hi, b_lo)
        nc.compile()
        nc.m = get_hw_module(nc.m)
        _NC_CACHE = nc
    return _NC_CACHE


def _split_hi_lo(x):
    """x: [HW, C] fp32 -> (hi, lo) bf16 with x ~= hi + lo."""
    hi = x.astype(BF16_NP)
    lo = (x - hi.astype(np.float32)).astype(BF16_NP)
    return hi, lo


def _pack_lhs(xT):
    """[C, HW] -> [128, HW/2]: rows 0:64 even m-tiles, rows 64:128 odd."""
    t = xT.reshape(C, M_PAIRS, 2, P)  # [c, pair, eo, j]
    return np.ascontiguousarray(t.transpose(2, 0, 1, 3).reshape(2 * C, M_PAIRS * P))


def _pack_rhs(xT):
    """[C, HW] -> [128, HW]: duplicate into both partition halves."""
    return np.ascontiguousarray(np.concatenate([xT, xT], axis=0))


def _prep_inputs(feature_A, feature_B):
    in_maps = []
    for i in range(B):
        A2 = np.ascontiguousarray(feature_A[i].reshape(HW, C), dtype=np.float32)
        B2 = np.ascontiguousarray(feature_B[i].reshape(HW, C), dtype=np.float32)
        ah, _ = _split_hi_lo(A2)
        bh, bl = _split_hi_lo(B2)
        in_maps.append(
            {
                "a_hi": _pack_lhs(np.ascontiguousarray(ah.T)),
                "b_hi": _pack_rhs(np.ascontiguousarray(bh.T)),
                "b_lo": _pack_rhs(np.ascontiguousarray(bl.T)),
            }
        )
    return in_maps


def _run(feature_A, feature_B, trace=False, **kwargs):
    feature_A = np.asarray(feature_A, dtype=np.float32)
    feature_B = np.asarray(feature_B, dtype=np.float32)
    assert feature_A.shape == (B, H, W, C), feature_A.shape
    assert feature_B.shape == (B, H, W, C), feature_B.shape

    nc = _build()
    in_maps = _prep_inputs(feature_A, feature_B)
    res = run_bass_kernel_spmd(nc, in_maps, list(range(B)), trace=trace, **kwargs)
    out = np.stack(
        [np.asarray(res.results[i]["out"]).astype(np.float32) for i in range(B)],
        axis=0,
    )
    return out.reshape(B, H, W, H, W), res


def kernel(feature_A, feature_B):
    out, _ = _run(feature_A, feature_B)
    return out



# revision 37
# speedup vs baseline: 1.0432x; 1.0432x over previous
"""Correlation network kernel for Trainium2.

corr[b,i,j,k,l] = sum_c A[b,i,j,c] * B[b,k,l,c]

Per batch b this is  A_b (2304x64) @ B_b^T (64x2304) -> 2304x2304.
Sharding: data-parallel over batch B=8 across the 8 NeuronCores; each core
computes one full 2304x2304 correlation matrix (21.2 MB fp32 out), so the
kernel is output-write bound (~358 GB/s HBM per core => ~60 us floor).

Device-side plan (per core):
  - Inputs arrive host-prepped: transposed to [C, HW] layout, bf16 hi/lo
    split. The kernel computes hi*hi + hi*lo (the lo*hi term is dropped)
    and emits the OUTPUT AS BF16, halving the dominant HBM write traffic
    (21.2 -> 10.6 MB/core; fro rel err ~2.3e-3, well inside the 2e-2
    gate; the host upcasts to fp32).
  - K=C=64 uses only half the 128-row PE array, so m-tiles are packed in
    pairs: even m-tiles occupy array rows 0-63, odd m-tiles rows 64-127.
    The two groups' matmuls run concurrently; B^T operands are duplicated
    into both partition halves so each group streams its own rows.
  - Inputs load via the two HWDGE rings (sync + scalar) right after the
    ~7 us Tile preamble; SWDGE/gpsimd was ~2 us slower to first byte.
  - Per (m-pair, 512-col n-tile): 4 bf16 matmuls into two PSUM banks,
    then narrow [128,512] PSUM->SBUF casts alternating between DVE and
    ACT (both engines ~3.2 us/pair -- the copy stream and the ~358 GB/s
    HBM write wire are the co-pacers of the kernel).
  - Output flushes: per m-tile one 512 KB chunk (cols 0:2048 = exactly
    4 KB/partition descriptors) + the 64 KB tail; even tiles on the SP
    ring, odd on the ACT ring so each engine issues one DMA per pair and
    waits only on its own side's copies.
"""

import numpy as np
import ml_dtypes

import concourse.bacc as bacc
import concourse.mybir as mybir
import concourse.tile as tile
from concourse.bass_interp import get_hw_module
from concourse.bass_utils import run_bass_kernel_spmd

B, H, W, C = 8, 48, 48, 64
HW = H * W  # 2304
P = 128
M_TILES = HW // P  # 18
M_PAIRS = M_TILES // 2  # 9
N_TILE = 512
FP32 = mybir.dt.float32
BF16 = mybir.dt.bfloat16
BF16_NP = ml_dtypes.bfloat16

N_SPLITS = []
_n0 = 0
while _n0 < HW:
    N_SPLITS.append((_n0, min(N_TILE, HW - _n0)))
    _n0 += N_TILE


def _corr_body(tc, out, a_hi, b_hi, b_lo):
    nc = tc.nc
    with (
        tc.tile_pool(name="ops", bufs=1) as op_pool,
        tc.tile_pool(name="ps", bufs=8, space="PSUM") as ps_pool,
        tc.tile_pool(name="outs", bufs=12) as out_pool,
    ):
        # lhsT operand: [128, 1152]; rows 0:64 = even m-tiles, 64:128 = odd
        ath = op_pool.tile([P, HW // 2], BF16)
        # rhs operands: [128, 2304]; rows 64:128 duplicate rows 0:64
        bth = op_pool.tile([P, HW], BF16)
        btl = op_pool.tile([P, HW], BF16)
        # Inputs ride the HWDGE rings (first byte ~0.6 us after the Tile
        # preamble vs ~9 us for the SWDGE/gpsimd path, and ~360 vs ~230
        # GB/s). Each dma_start costs ~0.6-0.7 us of issuing-engine time,
        # so split the issues across BOTH rings: sync takes the hh-term
        # operands (needed first), scalar takes the hl-term ones. Outputs
        # are issued later on the same rings (FIFO per ring, no conflict).
        for t, src, c0, c1 in [
            (ath, a_hi, 0, P),
            (bth, b_hi, 0, N_TILE),
            (ath, a_hi, P, HW // 2),
            (bth, b_hi, N_TILE, HW),
        ]:
            nc.sync.dma_start(out=t[:, c0:c1], in_=src[:, c0:c1])
        for t, src, c0, c1 in [
            (btl, b_lo, 0, N_TILE),
            (btl, b_lo, N_TILE, HW),
        ]:
            nc.scalar.dma_start(out=t[:, c0:c1], in_=src[:, c0:c1])

        for p in range(M_PAIRS):
            ot_e = out_pool.tile([P, HW], BF16, tag="ot")
            ot_o = out_pool.tile([P, HW], BF16, tag="ot")
            col = slice(p * P, (p + 1) * P)
            for ni, (n0, nsz) in enumerate(N_SPLITS):
                ps_e = ps_pool.tile([P, N_TILE], FP32, tag="ps")
                ps_o = ps_pool.tile([P, N_TILE], FP32, tag="ps")
                for k, (at, bt) in enumerate(((ath, bth), (ath, btl))):
                    st, sp = k == 0, k == 1
                    nc.tensor.matmul(
                        ps_e[:, :nsz],
                        at[0:64, col],
                        bt[0:64, n0 : n0 + nsz],
                        start=st,
                        stop=sp,
                    )
                    nc.tensor.matmul(
                        ps_o[:, :nsz],
                        at[64:128, col],
                        bt[64:128, n0 : n0 + nsz],
                        start=st,
                        stop=sp,
                    )
                # balance the narrow PSUM->SBUF copies across DVE and ACT
                # (narrow [128,512] copies keep the PSUM-recycle dependency
                # chain short; wide 2-bank copies measured slower overall)
                if ni % 2 == 0:
                    nc.vector.tensor_copy(ot_e[:, n0 : n0 + nsz], ps_e[:, :nsz])
                    nc.scalar.copy(ot_o[:, n0 : n0 + nsz], ps_o[:, :nsz])
                else:
                    nc.scalar.copy(ot_e[:, n0 : n0 + nsz], ps_e[:, :nsz])
                    nc.vector.tensor_copy(ot_o[:, n0 : n0 + nsz], ps_o[:, :nsz])
                # Flush 512 KB chunks (cols 0:2048 = exactly 4 KB/partition;
                # 4608 B patterns split 4096+512 and drop to ~205 GB/s) plus
                # the 64 KB tails. Even tiles drain on the SP ring, odd on
                # the ACT ring (a dma_start can only target the issuing
                # engine's own ring): each ring gets one DMA per pair
                # continuously, halving issue cost and blocking waits per
                # engine. The first pair flushes at ni=1 to start the
                # stream early.
                if p == 0:
                    sched = {1: 0, 3: 2 * N_TILE, 4: 4 * N_TILE}
                else:
                    sched = {3: 0, 4: 4 * N_TILE}
                if ni in sched:
                    c0 = sched[ni]
                    c1 = n0 + nsz
                    m_e, m_o = 2 * p, 2 * p + 1
                    nc.sync.dma_start(
                        out=out[m_e * P : (m_e + 1) * P, c0:c1],
                        in_=ot_e[:, c0:c1],
                    )
                    nc.scalar.dma_start(
                        out=out[m_o * P : (m_o + 1) * P, c0:c1],
                        in_=ot_o[:, c0:c1],
                    )


_NC_CACHE = None


def _build():
    global _NC_CACHE
    if _NC_CACHE is None:
        nc = bacc.Bacc(
            "TRN2",
            target_bir_lowering=False,
            debug=False,
            enable_asserts=False,
        )
        a_hi = nc.dram_tensor("a_hi", [P, HW // 2], BF16, kind="ExternalInput").ap()
        b_hi = nc.dram_tensor("b_hi", [P, HW], BF16, kind="ExternalInput").ap()
        b_lo = nc.dram_tensor("b_lo", [P, HW], BF16, kind="ExternalInput").ap()
        out = nc.dram_tensor("out", [HW, HW], BF16, kind="ExternalOutput").ap()
        with tile.TileContext(nc) as tc:
            _corr_body(tc, out, a_hi, b_hi, b_lo)
        nc.compile()
        nc.m = get_hw_module(nc.m)
        _NC_CACHE = nc
    return _NC_CACHE


def _split_hi_lo(x):
    """x: [HW, C] fp32 -> (hi, lo) bf16 with x ~= hi + lo."""
    hi = x.astype(BF16_NP)
    lo = (x - hi.astype(np.float32)).astype(BF16_NP)
    return hi, lo


def _pack_lhs(xT):
    """[C, HW] -> [128, HW/2]: rows 0:64 even m-tiles, rows 64:128 odd."""
    t = xT.reshape(C, M_PAIRS, 2, P)  # [c, pair, eo, j]
    return np.ascontiguousarray(t.transpose(2, 0, 1, 3).reshape(2 * C, M_PAIRS * P))


def _pack_rhs(xT):
    """[C, HW] -> [128, HW]: duplicate into both partition halves."""
    return np.ascontiguousarray(np.concatenate([xT, xT], axis=0))


def _prep_inputs(feature_A, feature_B):
    in_maps = []
    for i in range(B):
        A2 = np.ascontiguousarray(feature_A[i].reshape(HW, C), dtype=np.float32)
        B2 = np.ascontiguousarray(feature_B[i].reshape(HW, C), dtype=np.float32)
        ah, _ = _split_hi_lo(A2)
        bh, bl = _split_hi_lo(B2)
        in_maps.append(
            {
                "a_hi": _pack_lhs(np.ascontiguousarray(ah.T)),
                "b_hi": _pack_rhs(np.ascontiguousarray(bh.T)),
                "b_lo": _pack_rhs(np.ascontiguousarray(bl.T)),
            }
        )
    return in_maps


def _run(feature_A, feature_B, trace=False, **kwargs):
    feature_A = np.asarray(feature_A, dtype=np.float32)
    feature_B = np.asarray(feature_B, dtype=np.float32)
    assert feature_A.shape == (B, H, W, C), feature_A.shape
    assert feature_B.shape == (B, H, W, C), feature_B.shape

    nc = _build()
    in_maps = _prep_inputs(feature_A, feature_B)
    res = run_bass_kernel_spmd(nc, in_maps, list(range(B)), trace=trace, **kwargs)
    out = np.stack(
        [np.asarray(res.results[i]["out"]).astype(np.float32) for i in range(B)],
        axis=0,
    )
    return out.reshape(B, H, W, H, W), res


def kernel(feature_A, feature_B):
    out, _ = _run(feature_A, feature_B)
    return out



# revision 38
# speedup vs baseline: 1.0447x; 1.0015x over previous
"""Correlation network kernel for Trainium2.

corr[b,i,j,k,l] = sum_c A[b,i,j,c] * B[b,k,l,c]

Per batch b this is  A_b (2304x64) @ B_b^T (64x2304) -> 2304x2304.
Sharding: data-parallel over batch B=8 across the 8 NeuronCores; each core
computes one full 2304x2304 correlation matrix (21.2 MB fp32 out), so the
kernel is output-write bound (~358 GB/s HBM per core => ~60 us floor).

Device-side plan (per core):
  - Inputs arrive host-prepped: transposed to [C, HW] layout, bf16 hi/lo
    split. The kernel computes hi*hi + hi*lo (the lo*hi term is dropped)
    and emits the OUTPUT AS BF16, halving the dominant HBM write traffic
    (21.2 -> 10.6 MB/core; fro rel err ~2.3e-3, well inside the 2e-2
    gate; the host upcasts to fp32).
  - K=C=64 uses only half the 128-row PE array, so m-tiles are packed in
    pairs: even m-tiles occupy array rows 0-63, odd m-tiles rows 64-127.
    The two groups' matmuls run concurrently; B^T operands are duplicated
    into both partition halves so each group streams its own rows.
  - Inputs load via the two HWDGE rings (sync + scalar) right after the
    ~7 us Tile preamble; SWDGE/gpsimd was ~2 us slower to first byte.
  - Per (m-pair, 512-col n-tile): 4 bf16 matmuls into two PSUM banks,
    then narrow [128,512] PSUM->SBUF casts alternating between DVE and
    ACT (both engines ~3.2 us/pair -- the copy stream and the ~358 GB/s
    HBM write wire are the co-pacers of the kernel).
  - Output flushes: per m-tile one 512 KB chunk (cols 0:2048 = exactly
    4 KB/partition descriptors) + the 64 KB tail; even tiles on the SP
    ring, odd on the ACT ring so each engine issues one DMA per pair and
    waits only on its own side's copies.
"""

import numpy as np
import ml_dtypes

import concourse.bacc as bacc
import concourse.mybir as mybir
import concourse.tile as tile
from concourse.bass_interp import get_hw_module
from concourse.bass_utils import run_bass_kernel_spmd

B, H, W, C = 8, 48, 48, 64
HW = H * W  # 2304
P = 128
M_TILES = HW // P  # 18
M_PAIRS = M_TILES // 2  # 9
N_TILE = 512
FP32 = mybir.dt.float32
BF16 = mybir.dt.bfloat16
BF16_NP = ml_dtypes.bfloat16

N_SPLITS = []
_n0 = 0
while _n0 < HW:
    N_SPLITS.append((_n0, min(N_TILE, HW - _n0)))
    _n0 += N_TILE


def _corr_body(tc, out, a_hi, b_hi, b_lo):
    nc = tc.nc
    with (
        tc.tile_pool(name="ops", bufs=1) as op_pool,
        tc.tile_pool(name="ps", bufs=8, space="PSUM") as ps_pool,
        tc.tile_pool(name="outs", bufs=12) as out_pool,
    ):
        # lhsT operand: [128, 1152]; rows 0:64 = even m-tiles, 64:128 = odd
        ath = op_pool.tile([P, HW // 2], BF16)
        # rhs operands: [128, 2304]; rows 64:128 duplicate rows 0:64
        bth = op_pool.tile([P, HW], BF16)
        btl = op_pool.tile([P, HW], BF16)
        # Inputs ride the HWDGE rings (first byte ~0.6 us after the Tile
        # preamble vs ~9 us for the SWDGE/gpsimd path, and ~360 vs ~230
        # GB/s). Each dma_start costs ~0.6-0.7 us of issuing-engine time,
        # so split the issues across BOTH rings: sync takes the hh-term
        # operands (needed first), scalar takes the hl-term ones. Outputs
        # are issued later on the same rings (FIFO per ring, no conflict).
        for t, src, c0, c1 in [
            (ath, a_hi, 0, P),
            (bth, b_hi, 0, N_TILE),
            (ath, a_hi, P, HW // 2),
            (bth, b_hi, N_TILE, HW),
        ]:
            nc.sync.dma_start(out=t[:, c0:c1], in_=src[:, c0:c1])
        for t, src, c0, c1 in [
            (btl, b_lo, 0, N_TILE),
            (btl, b_lo, N_TILE, HW),
        ]:
            nc.scalar.dma_start(out=t[:, c0:c1], in_=src[:, c0:c1])

        for p in range(M_PAIRS):
            ot_e = out_pool.tile([P, HW], BF16, tag="ot")
            ot_o = out_pool.tile([P, HW], BF16, tag="ot")
            col = slice(p * P, (p + 1) * P)
            for ni, (n0, nsz) in enumerate(N_SPLITS):
                ps_e = ps_pool.tile([P, N_TILE], FP32, tag="ps")
                ps_o = ps_pool.tile([P, N_TILE], FP32, tag="ps")
                for k, (at, bt) in enumerate(((ath, bth), (ath, btl))):
                    st, sp = k == 0, k == 1
                    nc.tensor.matmul(
                        ps_e[:, :nsz],
                        at[0:64, col],
                        bt[0:64, n0 : n0 + nsz],
                        start=st,
                        stop=sp,
                    )
                    nc.tensor.matmul(
                        ps_o[:, :nsz],
                        at[64:128, col],
                        bt[64:128, n0 : n0 + nsz],
                        start=st,
                        stop=sp,
                    )
                # balance the narrow PSUM->SBUF copies across DVE and ACT
                # (narrow [128,512] copies keep the PSUM-recycle dependency
                # chain short; wide 2-bank copies measured slower overall)
                if ni % 2 == 0:
                    nc.vector.tensor_copy(ot_e[:, n0 : n0 + nsz], ps_e[:, :nsz])
                    nc.scalar.copy(ot_o[:, n0 : n0 + nsz], ps_o[:, :nsz])
                else:
                    nc.scalar.copy(ot_e[:, n0 : n0 + nsz], ps_e[:, :nsz])
                    nc.vector.tensor_copy(ot_o[:, n0 : n0 + nsz], ps_o[:, :nsz])
                # Flush 512 KB chunks (cols 0:2048 = exactly 4 KB/partition;
                # 4608 B patterns split 4096+512 and drop to ~205 GB/s) plus
                # the 64 KB tails. Even tiles drain on the SP ring, odd on
                # the ACT ring (a dma_start can only target the issuing
                # engine's own ring): each ring gets one DMA per pair
                # continuously, halving issue cost and blocking waits per
                # engine. The first pair flushes at ni=1 to start the
                # stream early.
                # Mid-stream pairs flush the full 2304-col row in ONE DMA
                # per tile (2 issues/pair instead of 4; the 4608 B/partition
                # descriptor split drops those DMAs to ~205 GB/s, but each
                # ring has duty slack against the ~3.9 us/pair production
                # cadence, and the issuing engines are the scarce resource).
                # First and last pairs stay chunked: p0 to start the stream
                # early, p8 so the final drain after the last copy is small.
                if p == 0:
                    sched = {1: 0, 3: 2 * N_TILE, 4: 4 * N_TILE}
                elif p == M_PAIRS - 1:
                    sched = {3: 0, 4: 4 * N_TILE}
                else:
                    sched = {4: 0}
                if ni in sched:
                    c0 = sched[ni]
                    c1 = n0 + nsz
                    m_e, m_o = 2 * p, 2 * p + 1
                    nc.sync.dma_start(
                        out=out[m_e * P : (m_e + 1) * P, c0:c1],
                        in_=ot_e[:, c0:c1],
                    )
                    nc.scalar.dma_start(
                        out=out[m_o * P : (m_o + 1) * P, c0:c1],
                        in_=ot_o[:, c0:c1],
                    )


_NC_CACHE = None


def _build():
    global _NC_CACHE
    if _NC_CACHE is None:
        nc = bacc.Bacc(
            "TRN2",
            target_bir_lowering=False,
            debug=False,
            enable_asserts=False,
        )
        a_hi = nc.dram_tensor("a_hi", [P, HW // 2], BF16, kind="ExternalInput").ap()
        b_hi = nc.dram_tensor("b_hi", [P, HW], BF16, kind="ExternalInput").ap()
        b_lo = nc.dram_tensor("b_lo", [P, HW], BF16, kind="ExternalInput").ap()
        out = nc.dram_tensor("out", [HW, HW], BF16, kind="ExternalOutput").ap()
        with tile.TileContext(nc) as tc:
            _corr_body(tc, out, a_hi, b_hi, b_lo)
        nc.compile()
        nc.m = get_hw_module(nc.m)
        _NC_CACHE = nc
    return _NC_CACHE


def _split_hi_lo(x):
    """x: [HW, C] fp32 -> (hi, lo) bf16 with x ~= hi + lo."""
    hi = x.astype(BF16_NP)
    lo = (x - hi.astype(np.float32)).astype(BF16_NP)
    return hi, lo


def _pack_lhs(xT):
    """[C, HW] -> [128, HW/2]: rows 0:64 even m-tiles, rows 64:128 odd."""
    t = xT.reshape(C, M_PAIRS, 2, P)  # [c, pair, eo, j]
    return np.ascontiguousarray(t.transpose(2, 0, 1, 3).reshape(2 * C, M_PAIRS * P))


def _pack_rhs(xT):
    """[C, HW] -> [128, HW]: duplicate into both partition halves."""
    return np.ascontiguousarray(np.concatenate([xT, xT], axis=0))


def _prep_inputs(feature_A, feature_B):
    in_maps = []
    for i in range(B):
        A2 = np.ascontiguousarray(feature_A[i].reshape(HW, C), dtype=np.float32)
        B2 = np.ascontiguousarray(feature_B[i].reshape(HW, C), dtype=np.float32)
        ah, _ = _split_hi_lo(A2)
        bh, bl = _split_hi_lo(B2)
        in_maps.append(
            {
                "a_hi": _pack_lhs(np.ascontiguousarray(ah.T)),
                "b_hi": _pack_rhs(np.ascontiguousarray(bh.T)),
                "b_lo": _pack_rhs(np.ascontiguousarray(bl.T)),
            }
        )
    return in_maps


def _run(feature_A, feature_B, trace=False, **kwargs):
    feature_A = np.asarray(feature_A, dtype=np.float32)
    feature_B = np.asarray(feature_B, dtype=np.float32)
    assert feature_A.shape == (B, H, W, C), feature_A.shape
    assert feature_B.shape == (B, H, W, C), feature_B.shape

    nc = _build()
    in_maps = _prep_inputs(feature_A, feature_B)
    res = run_bass_kernel_spmd(nc, in_maps, list(range(B)), trace=trace, **kwargs)
    out = np.stack(
        [np.asarray(res.results[i]["out"]).astype(np.float32) for i in range(B)],
        axis=0,
    )
    return out.reshape(B, H, W, H, W), res


def kernel(feature_A, feature_B):
    out, _ = _run(feature_A, feature_B)
    return out

